# revision 1
# baseline (speedup 1.0000x reference)
"""Distributed Trainium2 Bass kernel for blocked-sparse GNN message passing.

Computes  y = eps*diag*x + A @ (diag * mask * (A^T @ x)) * mask
where A is an NxN blocked-sparse matrix with per-edge 4x4 blocks.

Strategy (8 NeuronCores, two NEFF launches):
  Host: relabel nodes sorted by degree, tile 128 nodes/tile, round-robin
  tiles to cores (identical per-core program + balance), pad each tile to a
  shared degree schedule. Pre-permute boo_values into [p,i,t,s,j] blocks
  (transposed for pass 1), pre-gather x[row] (input-only data), precompute
  diag*mask and eps*x*diag tables.

  NEFF 1 (pass 1): per equal-degree chunk, bulk DMA + DVE broadcast-multiply
  + strided reduce -> per-core slice of w = diag*mask*(A^T x).
  Host: assemble w, apply the static gather w[col] into pass-2 slot layout.
  (Data-dependent DMA primitives — walrus DynamicAP indirect and the Ant
  dma_gather/dma_scatter_add ucode — are broken/hang on this runtime, so the
  one per-edge gather runs on host between the two launches.)
  NEFF 2 (pass 2): same multiply/reduce structure -> out = acc*mask +
  eps*x*diag per core slice; host unpermutes.
"""

import sys
import numpy as np

sys.path.insert(0, "/opt/trn_rl_repo")


def _install_axon_profile_hook():
    """Provide antenv.axon_hooks (absent in this container) so
    run_bass_kernel_spmd(trace=True) can capture NTFF profiles.  Reuses
    the boot shim's ctypes driver for libaxon_pjrt.so's NRT-profile ABI."""
    import types
    if "antenv.axon_hooks" in sys.modules:
        return

    def get_axon_ntff_profile_hook():
        try:
            sys.path.insert(0, "/root/.axon_site")
            from trn_agent_boot.trn_boot import _ntff_profile_via_ctypes
            return _ntff_profile_via_ctypes("/opt/axon/libaxon_pjrt.so")
        except Exception:
            return None

    m = types.ModuleType("antenv.axon_hooks")
    m.get_axon_ntff_profile_hook = get_axon_ntff_profile_hook
    sys.modules["antenv.axon_hooks"] = m


_install_axon_profile_hook()

P = 128          # SBUF partitions
NCORES = 8
D = 4            # block dim
EPSILON = 0.01
SLOT_CAP = 256   # max per-partition slots per chunk (bounds SBUF tile size)
USE_BF16 = True  # bf16 operands: halves boo DMA + enables DVE 2x TT mode


# ----------------------------------------------------------------------------
# Host-side planning
# ----------------------------------------------------------------------------

def _to_bf16(a):
    """Fast float32 -> bfloat16 (round-to-nearest-even), vectorized."""
    import ml_dtypes
    u = a.view(np.uint32)
    r = ((u >> 16) & 1) + 0x7FFF
    return ((u + r) >> 16).astype(np.uint16).view(ml_dtypes.bfloat16)


class PassPlan:
    """Static layout for one spmv pass (grouping endpoint = scatter node)."""

    def __init__(self, dst, n_nodes):
        # dst: [E] destination (grouping) node per edge
        n_pad = -(-n_nodes // (P * NCORES)) * (P * NCORES)
        deg = np.bincount(dst, minlength=n_pad).astype(np.int64)
        order = np.argsort(-deg, kind="stable")     # node ids, degree desc
        pos = np.empty(n_pad, dtype=np.int64)
        pos[order] = np.arange(n_pad)
        n_tiles = n_pad // P                        # global tiles (deg desc)
        self.tiles_per_core = n_tiles // NCORES
        # tile rank r -> core r%8, tile-index k=r//8
        tile_max = deg[order[::P]]                  # max degree of each tile
        dsch = np.maximum(tile_max[0::NCORES], 1)   # shared degree schedule
        dsch = ((dsch + 1) // 2) * 2                # even degrees: clean folds
        self.deg_sched = dsch.astype(np.int64)
        self.slots_pp = int(self.deg_sched.sum())   # per-partition slots
        # chunks: consecutive equal-degree tiles, capped at SLOT_CAP slots
        chunks = []
        k = 0
        K = self.tiles_per_core
        while k < K:
            d = int(self.deg_sched[k])
            t = 1
            while (k + t < K and self.deg_sched[k + t] == d
                   and (t + 1) * d <= SLOT_CAP):
                t += 1
            chunks.append((k, t, d))
            k += t
        self.chunks = chunks
        # per-tile slot base (within per-partition slot space)
        self.tile_base = np.concatenate([[0], np.cumsum(self.deg_sched)[:-1]])
        # per-edge coordinates
        q = pos[dst]                                # sorted position
        r = q // P
        self.p = (q % P).astype(np.int64)           # partition
        self.c = (r % NCORES).astype(np.int64)      # core
        self.k = (r // NCORES).astype(np.int64)     # tile idx within core
        # slot within node: rank of edge among its node's edges
        es = np.argsort(dst, kind="stable")
        cnt = np.bincount(dst, minlength=n_pad)
        starts = np.concatenate([[0], np.cumsum(cnt)[:-1]])
        s_sorted = np.arange(len(dst)) - starts[dst[es]]
        s = np.empty(len(dst), dtype=np.int64)
        s[es] = s_sorted
        self.s = s
        self.n_pad = n_pad
        self.pos = pos                              # node -> sorted position
        # node coords for table/aux layouts: node n -> (c,k,p)
        self.node_c = (pos // P) % NCORES
        self.node_k = (pos // P) // NCORES
        self.node_p = pos % P

    def edge_flat_slot(self):
        """Per-edge flat index u into the per-core chunked slot space:
        chunk blocks are laid [p][t][s], concatenated; u*4+j addresses the
        4-float record layout, 16*u the boo block layout."""
        base_pp = self.tile_base  # tile_base[k0] == chunk base_pp
        k0_of_k = np.zeros(self.tiles_per_core, dtype=np.int64)
        Td_of_k = np.zeros(self.tiles_per_core, dtype=np.int64)
        for (k0, T, d) in self.chunks:
            k0_of_k[k0:k0 + T] = k0
            Td_of_k[k0:k0 + T] = T * d
        k0e = k0_of_k[self.k]
        u = (P * base_pp[k0e]
             + self.p * Td_of_k[self.k]
             + (self.k - k0e) * self.deg_sched[self.k]
             + self.s)
        return u

    def aux_table(self, values, tiles_per_core):
        """Build per-core [128, K*4] aux arrays (p-major flat) from node-wise
        values [n_real, 4]: aux[c][p*K*4 + k*4 + i] = values[node at (c,k,p)]."""
        K = tiles_per_core
        out = np.zeros((NCORES, P * K * 4), dtype=np.float32)
        n_real = values.shape[0]
        c, k, p = self.node_c[:n_real], self.node_k[:n_real], self.node_p[:n_real]
        for i in range(4):
            out[c, p * (K * 4) + k * 4 + i] = values[:, i]
        return out


def plan_and_pack(x, edge_index, boo_values, mask, diag):
    """Host preprocessing: returns (meta, in_maps1, pass2 pack info)."""
    N = x.shape[0]
    E = edge_index.shape[1]
    row = np.asarray(edge_index[0], dtype=np.int64)
    col = np.asarray(edge_index[1], dtype=np.int64)
    x = np.asarray(x, dtype=np.float32)
    boo = np.asarray(boo_values, dtype=np.float32)
    diag = np.asarray(diag, dtype=np.float32)
    mask = np.asarray(mask, dtype=np.float32)

    pl1 = PassPlan(col, N)
    pl2 = PassPlan(row, N)
    K1, K2 = pl1.tiles_per_core, pl2.tiles_per_core
    S1, S2 = pl1.slots_pp, pl2.slots_pp
    NPC = K1 * P                      # nodes per core (table rows per rank)

    # ---- pass 1 arrays -----------------------------------------------------
    import ml_dtypes
    dt = ml_dtypes.bfloat16 if USE_BF16 else np.float32
    # vec operand compact [p, t, s, j]; expanded 16-wide on-chip (ScalarE)
    xg = np.zeros((NCORES, P * S1 * 4), dtype=np.float32)
    xr = x[row]                       # [E, 4]
    u1 = pl1.edge_flat_slot()
    for j in range(4):
        xg[pl1.c, u1 * 4 + j] = xr[:, j]
    del xr
    xg = _to_bf16(xg) if USE_BF16 else xg
    boo1 = np.zeros((NCORES, P * S1 * 16), dtype=np.float32)
    _scatter_boo(boo1, pl1, boo, transpose=True)
    boo1 = _to_bf16(boo1) if USE_BF16 else boo1
    dmv = (diag * mask).astype(np.float32)        # [N,4] diag*mask
    dm = pl1.aux_table(dmv, K1)
    # ---- pass 2 arrays -----------------------------------------------------
    boo2 = np.zeros((NCORES, P * S2 * 16), dtype=np.float32)
    _scatter_boo(boo2, pl2, boo, transpose=False)
    boo2 = _to_bf16(boo2) if USE_BF16 else boo2
    m2 = pl2.aux_table(np.broadcast_to(mask, (N, 4)).copy(), K2)
    epsd = pl2.aux_table((EPSILON * x * diag).astype(np.float32), K2)

    meta = dict(N=N, E=E, K1=K1, K2=K2, S1=S1, S2=S2, NPC=NPC,
                chunks1=pl1.chunks, chunks2=pl2.chunks)
    in_maps1 = [{"boo1": boo1[c], "xg": xg[c], "dm": dm[c]}
                for c in range(NCORES)]
    # pass-2 gather plan: wg slots get w_full[trow[col[e]]]
    trow = (pl1.node_c * NPC + pl1.node_k * P + pl1.node_p)
    gat_src = trow[col]
    # output unpermute info: node n -> (core, flat pos in out buffer)
    out_core = pl2.node_c[:N]
    out_off = pl2.node_p[:N] * (K2 * 4) + pl2.node_k[:N] * 4
    pack2 = dict(boo2=boo2, m2=m2, epsd=epsd, pl2=pl2, gat_src=gat_src,
                 out_core=out_core, out_off=out_off)
    return meta, in_maps1, pack2


def _scatter_boo(dst, pl, boo, transpose):
    """Scatter per-edge 4x4 blocks into the chunked [p][i][t][s][j] layout."""
    E = len(pl.c)
    k0_of_k = np.zeros(pl.tiles_per_core, dtype=np.int64)
    Td_of_k = np.zeros(pl.tiles_per_core, dtype=np.int64)
    for (k0, T, d) in pl.chunks:
        k0_of_k[k0:k0 + T] = k0
        Td_of_k[k0:k0 + T] = T * d
    k0e = k0_of_k[pl.k]
    Td = Td_of_k[pl.k]
    base = 16 * P * pl.tile_base[k0e]                      # chunk elem base
    ts = (pl.k - k0e) * pl.deg_sched[pl.k] + pl.s          # t*d + s
    ebase = base + pl.p * (4 * Td * 4) + ts * 4            # + i*(Td*4) + j
    istride = Td * 4
    CH = 1 << 19
    for lo in range(0, E, CH):
        hi = min(lo + CH, E)
        eb = ebase[lo:hi]
        ist = istride[lo:hi]
        cc = pl.c[lo:hi]
        blk = boo[lo:hi]
        for i in range(4):
            for j in range(4):
                v = blk[:, j, i] if transpose else blk[:, i, j]
                dst[cc, eb + i * ist + j] = v


# ----------------------------------------------------------------------------
# Device kernel builders
# ----------------------------------------------------------------------------

def _emit_pass(nc, tc, pool, chunks, boo_dram, vec_dram, acc_sb, mybir):
    """Shared pass structure: per chunk, prod = boo * vec16 (elementwise,
    both [p, i, t, s, j] layouts); reduce (s,j) per (i,tile) into
    acc_sb[:, k*4+i]."""
    dt = mybir.dt.bfloat16 if USE_BF16 else mybir.dt.float32
    base = 0
    for (k0, T, d) in chunks:
        F4 = T * d * 4
        Td = T * d
        boo_t = pool.tile([P, 4 * F4], dt, tag="boo_t")
        vec_t = pool.tile([P, F4], dt, tag="vec_t")
        exp_t = pool.tile([P, 4 * F4], dt, tag="exp_t")
        prod = pool.tile([P, 4 * F4], dt, tag="prod")
        b0 = 16 * P * base
        x0 = 4 * P * base
        nc.sync.dma_start(
            out=boo_t[:, :],
            in_=boo_dram.ap()[b0:b0 + P * 4 * F4].rearrange("(p f) -> p f", p=P))
        nc.sync.dma_start(
            out=vec_t[:, :],
            in_=vec_dram.ap()[x0:x0 + P * F4].rearrange("(p f) -> p f", p=P))
        # i-replicate vec on the (otherwise idle) scalar engine: boo layout
        # is [i, t, s, j] with i outermost, so the 4 replicas are contiguous
        # F4-sized planes — plain unit-stride copies
        for i in range(4):
            nc.scalar.activation(
                out=exp_t[:, i * F4:(i + 1) * F4],
                in_=vec_t[:, :],
                func=mybir.ActivationFunctionType.Copy)
        nc.vector.tensor_tensor(
            out=prod[:, :],
            in0=boo_t[:, :],
            in1=exp_t[:, :],
            op=mybir.AluOpType.mult)
        # fold the slot dim pairwise (bf16 2x adds) down to <=4 slots,
        # then one small tensor_reduce over the remaining (s,j)
        L = d
        blocks = prod[:, :].rearrange("p (b f) -> p b f", b=4 * T)
        while L > 4:
            if L % 2 == 1:
                # carry the odd tail slot into slot 0
                nc.vector.tensor_tensor(
                    out=blocks[:, :, :4], in0=blocks[:, :, :4],
                    in1=blocks[:, :, (L - 1) * 4:L * 4],
                    op=mybir.AluOpType.add)
                L -= 1
            h = L // 2
            nc.vector.tensor_tensor(
                out=blocks[:, :, :h * 4], in0=blocks[:, :, :h * 4],
                in1=blocks[:, :, h * 4:2 * h * 4],
                op=mybir.AluOpType.add)
            L = h
        nc.vector.tensor_reduce(
            out=acc_sb[:, k0 * 4:(k0 + T) * 4].rearrange(
                "p (t i) -> p i t", i=4),
            in_=prod[:, :].rearrange(
                "p (i t f) -> p i t f", i=4, t=T)[:, :, :, :L * 4],
            axis=mybir.AxisListType.X,
            op=mybir.AluOpType.add)
        base += T * d


def build_pass1(meta):
    import concourse.bacc as bacc
    import concourse.tile as tile
    from concourse import mybir

    K1, S1 = meta["K1"], meta["S1"]
    f32 = mybir.dt.float32
    dt = mybir.dt.bfloat16 if USE_BF16 else f32
    nc = bacc.Bacc("TRN2", target_bir_lowering=False, debug=False,
                   num_devices=NCORES)
    boo1 = nc.dram_tensor("boo1", [P * S1 * 16], dt, kind="ExternalInput")
    xg = nc.dram_tensor("xg", [P * S1 * 4], dt, kind="ExternalInput")
    dm = nc.dram_tensor("dm", [P * K1 * 4], f32, kind="ExternalInput")
    wout = nc.dram_tensor("wout", [P * K1 * 4], f32, kind="ExternalOutput")

    with tile.TileContext(nc) as tc:
        with tc.tile_pool(name="sb", bufs=3) as pool, \
             tc.tile_pool(name="acc", bufs=1) as apool:
            w_sb = apool.tile([P, K1 * 4], f32, tag="w_sb")
            _emit_pass(nc, tc, pool, meta["chunks1"], boo1, xg, w_sb, mybir)
            dm_t = apool.tile([P, K1 * 4], f32, tag="dm_t")
            nc.sync.dma_start(out=dm_t[:, :],
                              in_=dm.ap().rearrange("(p f) -> p f", p=P))
            nc.vector.tensor_tensor(out=w_sb[:, :], in0=w_sb[:, :],
                                    in1=dm_t[:, :], op=mybir.AluOpType.mult)
            nc.sync.dma_start(out=wout.ap().rearrange("(p f) -> p f", p=P),
                              in_=w_sb[:, :])
    nc.compile()
    return nc


def build_pass2(meta):
    import concourse.bacc as bacc
    import concourse.tile as tile
    from concourse import mybir

    K2, S2 = meta["K2"], meta["S2"]
    f32 = mybir.dt.float32
    dt = mybir.dt.bfloat16 if USE_BF16 else f32
    nc = bacc.Bacc("TRN2", target_bir_lowering=False, debug=False,
                   num_devices=NCORES)
    boo2 = nc.dram_tensor("boo2", [P * S2 * 16], dt, kind="ExternalInput")
    wg = nc.dram_tensor("wg", [P * S2 * 4], dt, kind="ExternalInput")
    m2 = nc.dram_tensor("m2", [P * K2 * 4], f32, kind="ExternalInput")
    epsd = nc.dram_tensor("epsd", [P * K2 * 4], f32, kind="ExternalInput")
    out = nc.dram_tensor("out", [P * K2 * 4], f32, kind="ExternalOutput")

    with tile.TileContext(nc) as tc:
        with tc.tile_pool(name="sb", bufs=3) as pool, \
             tc.tile_pool(name="acc", bufs=1) as apool:
            o_sb = apool.tile([P, K2 * 4], f32, tag="o_sb")
            _emit_pass(nc, tc, pool, meta["chunks2"], boo2, wg, o_sb, mybir)
            m2_t = apool.tile([P, K2 * 4], f32, tag="m2_t")
            ep_t = apool.tile([P, K2 * 4], f32, tag="ep_t")
            nc.sync.dma_start(out=m2_t[:, :],
                              in_=m2.ap().rearrange("(p f) -> p f", p=P))
            nc.sync.dma_start(out=ep_t[:, :],
                              in_=epsd.ap().rearrange("(p f) -> p f", p=P))
            nc.vector.tensor_tensor(out=o_sb[:, :], in0=o_sb[:, :],
                                    in1=m2_t[:, :], op=mybir.AluOpType.mult)
            nc.vector.tensor_tensor(out=o_sb[:, :], in0=o_sb[:, :],
                                    in1=ep_t[:, :], op=mybir.AluOpType.add)
            nc.sync.dma_start(out=out.ap().rearrange("(p f) -> p f", p=P),
                              in_=o_sb[:, :])
    nc.compile()
    return nc


# ----------------------------------------------------------------------------
# Entry point
# ----------------------------------------------------------------------------

_COMPILED = {}
last_results = None
last_exec_ns = None


def kernel(x, edge_index, boo_values, mask, diag):
    global last_results, last_exec_ns
    meta, in_maps1, pack2 = plan_and_pack(
        np.asarray(x), np.asarray(edge_index), np.asarray(boo_values),
        np.asarray(mask), np.asarray(diag))

    key = (meta["K1"], meta["K2"], meta["S1"], meta["S2"],
           tuple(meta["chunks1"]), tuple(meta["chunks2"]))
    if key not in _COMPILED:
        _COMPILED[key] = (build_pass1(meta), build_pass2(meta))
    nc1, nc2 = _COMPILED[key]

    import concourse.bass_utils as _bu
    _bu.upload_artifacts = lambda tmpdir: ""   # no bucket in this container
    run_bass_kernel_spmd = _bu.run_bass_kernel_spmd
    res1 = run_bass_kernel_spmd(nc1, in_maps1, core_ids=list(range(NCORES)))

    # host: assemble w and apply the static gather into pass-2 slot layout
    NPC, S2, K2, N = meta["NPC"], meta["S2"], meta["K2"], meta["N"]
    K1 = meta["K1"]
    w_full = np.empty((NCORES * NPC, 4), dtype=np.float32)
    for c in range(NCORES):
        ws = np.asarray(res1.results[c]["wout"]).reshape(P, K1, 4)
        # wout layout [p, k*4+i] -> table rows c*NPC + k*128 + p
        w_full[c * NPC:(c + 1) * NPC] = ws.transpose(1, 0, 2).reshape(NPC, 4)
    wg = np.zeros((NCORES, P * S2 * 4), dtype=np.float32)
    vals = w_full[pack2["gat_src"]]               # [E, 4]
    pl2 = pack2["pl2"]
    u2 = pl2.edge_flat_slot()
    for j in range(4):
        wg[pl2.c, u2 * 4 + j] = vals[:, j]
    wg = _to_bf16(wg) if USE_BF16 else wg

    in_maps2 = [{"boo2": pack2["boo2"][c], "wg": wg[c],
                 "m2": pack2["m2"][c], "epsd": pack2["epsd"][c]}
                for c in range(NCORES)]
    res2 = run_bass_kernel_spmd(nc2, in_maps2, core_ids=list(range(NCORES)))
    last_results = (res1, res2)
    t1 = res1.exec_time_ns
    t2 = res2.exec_time_ns
    last_exec_ns = (t1 + t2) if (t1 is not None and t2 is not None) else None

    outs = np.stack([np.asarray(res2.results[c]["out"]) for c in range(NCORES)])
    outs = outs.reshape(NCORES, -1)
    y = np.empty((N, 4), dtype=np.float32)
    oc, oo = pack2["out_core"], pack2["out_off"]
    for i in range(4):
        y[:, i] = outs[oc, oo + i]
    return y



# revision 6
# speedup vs baseline: 1.1963x; 1.1963x over previous
"""Distributed Trainium2 Bass kernel for blocked-sparse GNN message passing.

Computes  y = eps*diag*x + A @ (diag * mask * (A^T @ x)) * mask
where A is an NxN blocked-sparse matrix with per-edge 4x4 blocks.

Strategy (8 NeuronCores, two NEFF launches):
  Host: relabel nodes sorted by degree, tile 128 nodes/tile, round-robin
  tiles to cores (identical per-core program + balance), pad each tile to a
  shared degree schedule. Pre-permute boo_values into [p,i,t,s,j] blocks
  (transposed for pass 1), pre-gather x[row] (input-only data), precompute
  diag*mask and eps*x*diag tables.

  NEFF 1 (pass 1): per equal-degree chunk, bulk DMA + DVE broadcast-multiply
  + strided reduce -> per-core slice of w = diag*mask*(A^T x).
  Host: assemble w, apply the static gather w[col] into pass-2 slot layout.
  (Data-dependent DMA primitives — walrus DynamicAP indirect and the Ant
  dma_gather/dma_scatter_add ucode — are broken/hang on this runtime, so the
  one per-edge gather runs on host between the two launches.)
  NEFF 2 (pass 2): same multiply/reduce structure -> out = acc*mask +
  eps*x*diag per core slice; host unpermutes.
"""

import sys
import numpy as np

sys.path.insert(0, "/opt/trn_rl_repo")


def _install_axon_profile_hook():
    """Provide antenv.axon_hooks (absent in this container) so
    run_bass_kernel_spmd(trace=True) can capture NTFF profiles.  Reuses
    the boot shim's ctypes driver for libaxon_pjrt.so's NRT-profile ABI."""
    import types
    if "antenv.axon_hooks" in sys.modules:
        return

    def get_axon_ntff_profile_hook():
        try:
            sys.path.insert(0, "/root/.axon_site")
            from trn_agent_boot.trn_boot import _ntff_profile_via_ctypes
            return _ntff_profile_via_ctypes("/opt/axon/libaxon_pjrt.so")
        except Exception:
            return None

    m = types.ModuleType("antenv.axon_hooks")
    m.get_axon_ntff_profile_hook = get_axon_ntff_profile_hook
    sys.modules["antenv.axon_hooks"] = m


_install_axon_profile_hook()

P = 128          # SBUF partitions
NCORES = 8
D = 4            # block dim
EPSILON = 0.01
SLOT_CAP = 256   # max per-partition slots per chunk (bounds SBUF tile size)
USE_BF16 = True  # bf16 operands: halves boo DMA + enables DVE 2x TT mode


# ----------------------------------------------------------------------------
# Host-side planning
# ----------------------------------------------------------------------------

def _to_bf16(a):
    """Fast float32 -> bfloat16 (round-to-nearest-even), vectorized."""
    import ml_dtypes
    u = a.view(np.uint32)
    r = ((u >> 16) & 1) + 0x7FFF
    return ((u + r) >> 16).astype(np.uint16).view(ml_dtypes.bfloat16)


class PassPlan:
    """Static layout for one spmv pass (grouping endpoint = scatter node)."""

    def __init__(self, dst, n_nodes):
        # dst: [E] destination (grouping) node per edge
        n_pad = -(-n_nodes // (P * NCORES)) * (P * NCORES)
        deg = np.bincount(dst, minlength=n_pad).astype(np.int64)
        order = np.argsort(-deg, kind="stable")     # node ids, degree desc
        pos = np.empty(n_pad, dtype=np.int64)
        pos[order] = np.arange(n_pad)
        n_tiles = n_pad // P                        # global tiles (deg desc)
        self.tiles_per_core = n_tiles // NCORES
        # tile rank r -> core r%8, tile-index k=r//8
        tile_max = deg[order[::P]]                  # max degree of each tile
        dsch = np.maximum(tile_max[0::NCORES], 1)   # shared degree schedule
        dsch = ((dsch + 1) // 2) * 2                # even degrees: clean folds
        self.deg_sched = dsch.astype(np.int64)
        self.slots_pp = int(self.deg_sched.sum())   # per-partition slots
        # chunks: consecutive equal-degree tiles, capped at SLOT_CAP slots
        chunks = []
        k = 0
        K = self.tiles_per_core
        while k < K:
            d = int(self.deg_sched[k])
            t = 1
            while (k + t < K and self.deg_sched[k + t] == d
                   and (t + 1) * d <= SLOT_CAP):
                t += 1
            chunks.append((k, t, d))
            k += t
        self.chunks = chunks
        # per-tile slot base (within per-partition slot space)
        self.tile_base = np.concatenate([[0], np.cumsum(self.deg_sched)[:-1]])
        # per-edge coordinates
        q = pos[dst]                                # sorted position
        r = q // P
        self.p = (q % P).astype(np.int64)           # partition
        self.c = (r % NCORES).astype(np.int64)      # core
        self.k = (r // NCORES).astype(np.int64)     # tile idx within core
        # slot within node: rank of edge among its node's edges
        es = np.argsort(dst, kind="stable")
        cnt = np.bincount(dst, minlength=n_pad)
        starts = np.concatenate([[0], np.cumsum(cnt)[:-1]])
        s_sorted = np.arange(len(dst)) - starts[dst[es]]
        s = np.empty(len(dst), dtype=np.int64)
        s[es] = s_sorted
        self.s = s
        self.n_pad = n_pad
        self.pos = pos                              # node -> sorted position
        # node coords for table/aux layouts: node n -> (c,k,p)
        self.node_c = (pos // P) % NCORES
        self.node_k = (pos // P) // NCORES
        self.node_p = pos % P

    def edge_flat_slot(self):
        """Per-edge flat index u into the per-core chunked slot space:
        chunk blocks are laid [p][t][s], concatenated; u*4+j addresses the
        4-float record layout, 16*u the boo block layout."""
        base_pp = self.tile_base  # tile_base[k0] == chunk base_pp
        k0_of_k = np.zeros(self.tiles_per_core, dtype=np.int64)
        Td_of_k = np.zeros(self.tiles_per_core, dtype=np.int64)
        for (k0, T, d) in self.chunks:
            k0_of_k[k0:k0 + T] = k0
            Td_of_k[k0:k0 + T] = T * d
        k0e = k0_of_k[self.k]
        u = (P * base_pp[k0e]
             + self.p * Td_of_k[self.k]
             + (self.k - k0e) * self.deg_sched[self.k]
             + self.s)
        return u

    def aux_table(self, values, tiles_per_core):
        """Build per-core [128, K*4] aux arrays (p-major flat) from node-wise
        values [n_real, 4]: aux[c][p*K*4 + k*4 + i] = values[node at (c,k,p)]."""
        K = tiles_per_core
        out = np.zeros((NCORES, P * K * 4), dtype=np.float32)
        n_real = values.shape[0]
        c, k, p = self.node_c[:n_real], self.node_k[:n_real], self.node_p[:n_real]
        for i in range(4):
            out[c, p * (K * 4) + k * 4 + i] = values[:, i]
        return out


def plan_and_pack(x, edge_index, boo_values, mask, diag):
    """Host preprocessing: returns (meta, in_maps1, pass2 pack info)."""
    N = x.shape[0]
    E = edge_index.shape[1]
    row = np.asarray(edge_index[0], dtype=np.int64)
    col = np.asarray(edge_index[1], dtype=np.int64)
    x = np.asarray(x, dtype=np.float32)
    boo = np.asarray(boo_values, dtype=np.float32)
    diag = np.asarray(diag, dtype=np.float32)
    mask = np.asarray(mask, dtype=np.float32)

    pl1 = PassPlan(col, N)
    pl2 = PassPlan(row, N)
    K1, K2 = pl1.tiles_per_core, pl2.tiles_per_core
    S1, S2 = pl1.slots_pp, pl2.slots_pp
    NPC = K1 * P                      # nodes per core (table rows per rank)

    # ---- pass 1 arrays -----------------------------------------------------
    import ml_dtypes
    dt = ml_dtypes.bfloat16 if USE_BF16 else np.float32
    # vec operand compact [p, t, s, j]; expanded 16-wide on-chip (ScalarE)
    xg = np.zeros((NCORES, P * S1 * 4), dtype=np.float32)
    xr = x[row]                       # [E, 4]
    u1 = pl1.edge_flat_slot()
    for j in range(4):
        xg[pl1.c, u1 * 4 + j] = xr[:, j]
    del xr
    xg = _to_bf16(xg) if USE_BF16 else xg
    boo1 = np.zeros((NCORES, P * S1 * 16), dtype=np.float32)
    _scatter_boo(boo1, pl1, boo, transpose=True)
    boo1 = _to_bf16(boo1) if USE_BF16 else boo1
    dmv = (diag * mask).astype(np.float32)        # [N,4] diag*mask
    dm = pl1.aux_table(dmv, K1)
    # ---- pass 2 arrays -----------------------------------------------------
    boo2 = np.zeros((NCORES, P * S2 * 16), dtype=np.float32)
    _scatter_boo(boo2, pl2, boo, transpose=False)
    boo2 = _to_bf16(boo2) if USE_BF16 else boo2
    m2 = pl2.aux_table(np.broadcast_to(mask, (N, 4)).copy(), K2)
    epsd = pl2.aux_table((EPSILON * x * diag).astype(np.float32), K2)

    meta = dict(N=N, E=E, K1=K1, K2=K2, S1=S1, S2=S2, NPC=NPC,
                chunks1=pl1.chunks, chunks2=pl2.chunks)
    in_maps1 = [{"boo1": boo1[c], "xg": xg[c], "dm": dm[c]}
                for c in range(NCORES)]
    # pass-2 gather plan: wg slots get w_full[trow[col[e]]]
    trow = (pl1.node_c * NPC + pl1.node_k * P + pl1.node_p)
    gat_src = trow[col]
    # output unpermute info: node n -> (core, flat pos in out buffer)
    out_core = pl2.node_c[:N]
    out_off = pl2.node_p[:N] * (K2 * 4) + pl2.node_k[:N] * 4
    pack2 = dict(boo2=boo2, m2=m2, epsd=epsd, pl2=pl2, gat_src=gat_src,
                 out_core=out_core, out_off=out_off)
    return meta, in_maps1, pack2


def _scatter_boo(dst, pl, boo, transpose):
    """Scatter per-edge 4x4 blocks into the chunked [p][i][t][s][j] layout."""
    E = len(pl.c)
    k0_of_k = np.zeros(pl.tiles_per_core, dtype=np.int64)
    Td_of_k = np.zeros(pl.tiles_per_core, dtype=np.int64)
    for (k0, T, d) in pl.chunks:
        k0_of_k[k0:k0 + T] = k0
        Td_of_k[k0:k0 + T] = T * d
    k0e = k0_of_k[pl.k]
    Td = Td_of_k[pl.k]
    base = 16 * P * pl.tile_base[k0e]                      # chunk elem base
    ts = (pl.k - k0e) * pl.deg_sched[pl.k] + pl.s          # t*d + s
    ebase = base + pl.p * (4 * Td * 4) + ts * 4            # + i*(Td*4) + j
    istride = Td * 4
    CH = 1 << 19
    for lo in range(0, E, CH):
        hi = min(lo + CH, E)
        eb = ebase[lo:hi]
        ist = istride[lo:hi]
        cc = pl.c[lo:hi]
        blk = boo[lo:hi]
        for i in range(4):
            for j in range(4):
                v = blk[:, j, i] if transpose else blk[:, i, j]
                dst[cc, eb + i * ist + j] = v


# ----------------------------------------------------------------------------
# Device kernel builders
# ----------------------------------------------------------------------------

def _emit_pass(nc, tc, pool, psum_pool, chunks, boo_dram, vec_dram, acc_sb,
               ident_t, mybir):
    """Shared pass structure: per chunk,
      1) DVE: prod[p,i,t,s,j] = boo[p,i,t,s,j] * vec[p,t,s,j] (stride-0
         broadcast of vec over i; bf16 2x mode, one instruction)
      2) PE: identity-weight matmuls copy-accumulate the s-slices into a
         PSUM region psum[p, it, s%g, j] (fp32 accumulate, ~1 col/cycle)
      3) DVE: one small tensor_reduce over (s%g, j) from PSUM into
         acc_sb[:, k*4+i]."""
    from concourse.bass import broadcast_tensor_aps
    dt = mybir.dt.bfloat16 if USE_BF16 else mybir.dt.float32
    f32 = mybir.dt.float32
    base = 0
    for (k0, T, d) in chunks:
        F4 = T * d * 4
        Td = T * d
        g = min(d, max(1, 512 // (16 * T)))   # s-slices per matmul
        nseg = -(-d // g)
        boo_t = pool.tile([P, 4 * F4], dt, tag="boo_t")
        vec_t = pool.tile([P, F4], dt, tag="vec_t")
        prod = pool.tile([P, 4 * F4], dt, tag="prod")
        psum_t = psum_pool.tile([P, 16 * T * g], f32, tag="ps")
        b0 = 16 * P * base
        x0 = 4 * P * base
        nc.sync.dma_start(
            out=boo_t[:, :],
            in_=boo_dram.ap()[b0:b0 + P * 4 * F4].rearrange("(p f) -> p f", p=P))
        nc.sync.dma_start(
            out=vec_t[:, :],
            in_=vec_dram.ap()[x0:x0 + P * F4].rearrange("(p f) -> p f", p=P))
        in0 = boo_t[:, :].rearrange("p (i f) -> p i f", i=4)
        in1 = vec_t[:, :].rearrange("p (one f) -> p one f", one=1)
        in0b, in1b = broadcast_tensor_aps(in0, in1)
        nc.vector.tensor_tensor(
            out=prod[:, :].rearrange("p (i f) -> p i f", i=4),
            in0=in0b, in1=in1b, op=mybir.AluOpType.mult)
        rhs_all = prod[:, :].rearrange(
            "p (i t s j) -> p s (i t) j", i=4, t=T, s=d, j=4)
        out_all = psum_t[:, :].rearrange(
            "p (it s j) -> p s it j", it=4 * T, s=g, j=4)
        for m in range(nseg):
            s0 = m * g
            gs = min(g, d - s0)
            nc.tensor.matmul(
                out=out_all[:, :gs],
                lhsT=ident_t[:, :],
                rhs=rhs_all[:, s0:s0 + gs],
                start=(m == 0), stop=(m == nseg - 1))
        nc.vector.tensor_reduce(
            out=acc_sb[:, k0 * 4:(k0 + T) * 4].rearrange(
                "p (t i) -> p i t", i=4),
            in_=psum_t[:, :].rearrange("p (it sj) -> p it sj",
                                       it=4 * T, sj=4 * g),
            axis=mybir.AxisListType.X,
            op=mybir.AluOpType.add)
        base += Td


def build_pass1(meta):
    import concourse.bacc as bacc
    import concourse.tile as tile
    from concourse import mybir

    K1, S1 = meta["K1"], meta["S1"]
    f32 = mybir.dt.float32
    dt = mybir.dt.bfloat16 if USE_BF16 else f32
    nc = bacc.Bacc("TRN2", target_bir_lowering=False, debug=False,
                   num_devices=NCORES)
    boo1 = nc.dram_tensor("boo1", [P * S1 * 16], dt, kind="ExternalInput")
    xg = nc.dram_tensor("xg", [P * S1 * 4], dt, kind="ExternalInput")
    dm = nc.dram_tensor("dm", [P * K1 * 4], f32, kind="ExternalInput")
    ident = nc.dram_tensor("ident", [P * P], dt, kind="ExternalInput")
    wout = nc.dram_tensor("wout", [P * K1 * 4], f32, kind="ExternalOutput")

    with tile.TileContext(nc) as tc:
        with tc.tile_pool(name="sb", bufs=3) as pool, \
             tc.tile_pool(name="ps", bufs=2, space="PSUM") as psum_pool, \
             tc.tile_pool(name="acc", bufs=1) as apool:
            ident_t = apool.tile([P, P], dt, tag="ident_t")
            nc.sync.dma_start(out=ident_t[:, :],
                              in_=ident.ap().rearrange("(p f) -> p f", p=P))
            w_sb = apool.tile([P, K1 * 4], f32, tag="w_sb")
            _emit_pass(nc, tc, pool, psum_pool, meta["chunks1"], boo1, xg,
                       w_sb, ident_t, mybir)
            dm_t = apool.tile([P, K1 * 4], f32, tag="dm_t")
            nc.sync.dma_start(out=dm_t[:, :],
                              in_=dm.ap().rearrange("(p f) -> p f", p=P))
            nc.vector.tensor_tensor(out=w_sb[:, :], in0=w_sb[:, :],
                                    in1=dm_t[:, :], op=mybir.AluOpType.mult)
            nc.sync.dma_start(out=wout.ap().rearrange("(p f) -> p f", p=P),
                              in_=w_sb[:, :])
    nc.compile()
    return nc


def build_pass2(meta):
    import concourse.bacc as bacc
    import concourse.tile as tile
    from concourse import mybir

    K2, S2 = meta["K2"], meta["S2"]
    f32 = mybir.dt.float32
    dt = mybir.dt.bfloat16 if USE_BF16 else f32
    nc = bacc.Bacc("TRN2", target_bir_lowering=False, debug=False,
                   num_devices=NCORES)
    boo2 = nc.dram_tensor("boo2", [P * S2 * 16], dt, kind="ExternalInput")
    wg = nc.dram_tensor("wg", [P * S2 * 4], dt, kind="ExternalInput")
    m2 = nc.dram_tensor("m2", [P * K2 * 4], f32, kind="ExternalInput")
    epsd = nc.dram_tensor("epsd", [P * K2 * 4], f32, kind="ExternalInput")
    ident = nc.dram_tensor("ident", [P * P], dt, kind="ExternalInput")
    out = nc.dram_tensor("out", [P * K2 * 4], f32, kind="ExternalOutput")

    with tile.TileContext(nc) as tc:
        with tc.tile_pool(name="sb", bufs=3) as pool, \
             tc.tile_pool(name="ps", bufs=2, space="PSUM") as psum_pool, \
             tc.tile_pool(name="acc", bufs=1) as apool:
            ident_t = apool.tile([P, P], dt, tag="ident_t")
            nc.sync.dma_start(out=ident_t[:, :],
                              in_=ident.ap().rearrange("(p f) -> p f", p=P))
            o_sb = apool.tile([P, K2 * 4], f32, tag="o_sb")
            _emit_pass(nc, tc, pool, psum_pool, meta["chunks2"], boo2, wg,
                       o_sb, ident_t, mybir)
            m2_t = apool.tile([P, K2 * 4], f32, tag="m2_t")
            ep_t = apool.tile([P, K2 * 4], f32, tag="ep_t")
            nc.sync.dma_start(out=m2_t[:, :],
                              in_=m2.ap().rearrange("(p f) -> p f", p=P))
            nc.sync.dma_start(out=ep_t[:, :],
                              in_=epsd.ap().rearrange("(p f) -> p f", p=P))
            nc.vector.tensor_tensor(out=o_sb[:, :], in0=o_sb[:, :],
                                    in1=m2_t[:, :], op=mybir.AluOpType.mult)
            nc.vector.tensor_tensor(out=o_sb[:, :], in0=o_sb[:, :],
                                    in1=ep_t[:, :], op=mybir.AluOpType.add)
            nc.sync.dma_start(out=out.ap().rearrange("(p f) -> p f", p=P),
                              in_=o_sb[:, :])
    nc.compile()
    return nc


# ----------------------------------------------------------------------------
# Entry point
# ----------------------------------------------------------------------------

_COMPILED = {}
last_results = None
last_exec_ns = None


def kernel(x, edge_index, boo_values, mask, diag):
    global last_results, last_exec_ns
    meta, in_maps1, pack2 = plan_and_pack(
        np.asarray(x), np.asarray(edge_index), np.asarray(boo_values),
        np.asarray(mask), np.asarray(diag))

    key = (meta["K1"], meta["K2"], meta["S1"], meta["S2"],
           tuple(meta["chunks1"]), tuple(meta["chunks2"]))
    if key not in _COMPILED:
        _COMPILED[key] = (build_pass1(meta), build_pass2(meta))
    nc1, nc2 = _COMPILED[key]

    import concourse.bass_utils as _bu
    _bu.upload_artifacts = lambda tmpdir: ""   # no bucket in this container
    run_bass_kernel_spmd = _bu.run_bass_kernel_spmd
    ident_np = np.eye(P, dtype=np.float32).reshape(-1)
    ident_np = _to_bf16(ident_np) if USE_BF16 else ident_np
    for im in in_maps1:
        im["ident"] = ident_np
    res1 = run_bass_kernel_spmd(nc1, in_maps1, core_ids=list(range(NCORES)))

    # host: assemble w and apply the static gather into pass-2 slot layout
    NPC, S2, K2, N = meta["NPC"], meta["S2"], meta["K2"], meta["N"]
    K1 = meta["K1"]
    w_full = np.empty((NCORES * NPC, 4), dtype=np.float32)
    for c in range(NCORES):
        ws = np.asarray(res1.results[c]["wout"]).reshape(P, K1, 4)
        # wout layout [p, k*4+i] -> table rows c*NPC + k*128 + p
        w_full[c * NPC:(c + 1) * NPC] = ws.transpose(1, 0, 2).reshape(NPC, 4)
    wg = np.zeros((NCORES, P * S2 * 4), dtype=np.float32)
    vals = w_full[pack2["gat_src"]]               # [E, 4]
    pl2 = pack2["pl2"]
    u2 = pl2.edge_flat_slot()
    for j in range(4):
        wg[pl2.c, u2 * 4 + j] = vals[:, j]
    wg = _to_bf16(wg) if USE_BF16 else wg

    in_maps2 = [{"boo2": pack2["boo2"][c], "wg": wg[c],
                 "m2": pack2["m2"][c], "epsd": pack2["epsd"][c],
                 "ident": ident_np}
                for c in range(NCORES)]
    res2 = run_bass_kernel_spmd(nc2, in_maps2, core_ids=list(range(NCORES)))
    last_results = (res1, res2)
    t1 = res1.exec_time_ns
    t2 = res2.exec_time_ns
    last_exec_ns = (t1 + t2) if (t1 is not None and t2 is not None) else None

    outs = np.stack([np.asarray(res2.results[c]["out"]) for c in range(NCORES)])
    outs = outs.reshape(NCORES, -1)
    y = np.empty((N, 4), dtype=np.float32)
    oc, oo = pack2["out_core"], pack2["out_off"]
    for i in range(4):
        y[:, i] = outs[oc, oo + i]
    return y

